# revision 96
# baseline (speedup 1.0000x reference)
"""Trainium2 Bass kernel for nn_AttentionLayer (B=4, C=256, N=4096, CR=32).

Sharding: 8 cores = (batch b in 0..3) x (query-half ih in 0..1).
Each core receives x[b] rotated so its own query half sits at columns
0..2047 (softmax is invariant to key order, so the rotation is exact);
it computes out[b][:, ih*2048:(ih+1)*2048] and the host reassembles.

Per-core algorithm:
  - g conv (keys + gbv bias-correction row) in f32r: [33,C] @ x -> g
  - h^T conv: lhsT = x chunk (stationary), rhs = Wq^T bf16 (moving,
    ap=32) -> h^T j-tiles directly in [j, 32] layout (no PE transpose)
  - f conv (queries, own half) in f32r; f/g stored as fp8e4m3 in SBUF
  - scores via fp8 DoubleRow matmul with a broadcast (stride-0) slot
    dim on both operands: psum = 2*(g_aug^T f_aug), 0.5 cycles/row.
    The 2x is undone inside exp (scale=0.5).
  - exp split across three engines: ACT native Exp; DVE/Pool compute
    Schraudolph bits = round(s*64/ln2 + B) written as int16 == bf16.
  - mm2 swapped: lhsT = eb (stationary bf16), rhs = hpt [j,33] bf16
    (moving, ap=33) accumulating num^T/den in [i, 33] psum chunks.
  - tail per i-chunk: rden = 1/den (per-partition), att^T = po*rden
    (bf16; row 32 becomes den*rden ~= 1 and doubles as the out-conv
    bias-ones row), PE transpose (bf16 identity), out conv
    (gamma*Wo^T | bias row), residual add fused into PSUM->SBUF copy.
"""

import os
import numpy as np

B, C, N = 4, 256, 4096
CR = 32
NH = N // 2          # queries per core
G = 512              # i-group width
NCORES = 8

NJT = N // 128       # 32 j-tiles
NIG = NH // G        # 4 i-groups
SUP = int(os.environ.get("KN_SUP", "2"))   # max j-tiles per stage
# per-i-group supers list (sums to NJT)
SUPERS = [SUP] * (NJT // SUP) + ([NJT % SUP] if NJT % SUP else [])
NST = len(SUPERS)    # stages per i-group
JT0 = [sum(SUPERS[:i]) for i in range(NST)]

# xw (f32r) layout: true-f32r data, DMA TF32 rounding is acceptable
W_WG = 0             # g conv lhsT   [128, 66]  (2 chunks x 33)
W_P0 = 66            # x piece 0     [128, 1024]
W_WF = 1090          # f conv lhsT   [128, 64]  (2 chunks x 32)
W_WQ = 1154          # wqt f32r      [128, 2x32]
W_IDR = 1218         # idr f32r identity [128, 128]
W_CON = 1346         # end of consts
WTOT = W_CON + 7 * 1024
# xc (f32, bit-exact DMA) layout: bit-packed bf16 constants
C_WO = 0             # wotb bf16 [33, 256] packed as u32 [33, 128]
C_ID = 128           # idm128 bf16 [128, 128] packed as u32 [128, 64]
C_TOT = 192

_CACHE = {}


def build_program():
    import concourse.bacc as bacc
    import concourse.mybir as mybir
    from concourse.tile import TileContext

    dt = mybir.dt
    f32 = dt.float32
    f32r = dt.float32r
    bf16 = dt.bfloat16
    fp8 = dt.float8e4
    i16 = dt.int16
    Exp = mybir.ActivationFunctionType.Exp
    add = mybir.AluOpType.add
    mult = mybir.AluOpType.mult
    DR = mybir.MatmulPerfMode.DoubleRow

    A_SCH = 64.0 / np.log(2.0)          # schraudolph slope on 2s input
    B_SCH = 127.0 * 128.0 - 7.0 + 0.5   # bias incl +0.5 for truncation

    nc = bacc.Bacc("TRN2", target_bir_lowering=False, debug=False,
                   num_devices=NCORES)

    # xw is f32r (DMA rounds to TF32 - fine for x and real weights); the
    # bit-packed bf16 constants ride in xc as plain f32 (bit-exact DMA).
    xw = nc.dram_tensor("xw", [128, WTOT], f32r, kind="ExternalInput").ap()
    xc = nc.dram_tensor("xc", [128, C_TOT], f32, kind="ExternalInput").ap()
    res = nc.dram_tensor("res", [C, NH], f32, kind="ExternalOutput").ap()
    DBG = os.environ.get("KN_DEBUG", "") == "1"
    if DBG:
        dbg_g = nc.dram_tensor("dbg_g", [33, N], f32, kind="ExternalOutput").ap()
        dbg_f = nc.dram_tensor("dbg_f", [33, G], f32, kind="ExternalOutput").ap()
        dbg_h = nc.dram_tensor("dbg_h", [128, NJT * 33], f32, kind="ExternalOutput").ap()
        dbg_eb = nc.dram_tensor("dbg_eb", [128, SUP * G], f32, kind="ExternalOutput").ap()
        dbg_po = nc.dram_tensor("dbg_po", [128, 136], f32, kind="ExternalOutput").ap()
        dbg_asc = nc.dram_tensor("dbg_asc", [128, 136], f32, kind="ExternalOutput").ap()
        dbg_att = nc.dram_tensor("dbg_att", [33, 512], f32, kind="ExternalOutput").ap()
        dbg_wot = nc.dram_tensor("dbg_wot", [33, 256], f32, kind="ExternalOutput").ap()
        dbg_op = nc.dram_tensor("dbg_op", [128, 1024], f32, kind="ExternalOutput").ap()

    # exp engine schedule per stage: A=ACT native exp, D=DVE schraudolph.
    # ig0 keeps DVE mostly free for conv copies; later igs alternate more.
    sched = os.environ.get(
        "KN_EXP", "A" * NST + "AADAADAADAA"[:NST] * 3 if NST == 11
        else "A" * 16 + "AADADAADADAADADA" * 3)
    assert len(sched) >= NIG * NST

    with TileContext(nc) as tc:
        with (
            tc.tile_pool(name="const", bufs=1) as cpool,
            tc.tile_pool(name="eb", bufs=6) as epool,
            tc.tile_pool(name="small", bufs=2) as spool,
            tc.tile_pool(name="resp", bufs=2) as rpool,
            tc.tile_pool(name="psS", bufs=(3 if SUP == 2 else 2), space="PSUM") as psS,
            tc.tile_pool(name="psC", bufs=1, space="PSUM") as psC,
            tc.tile_pool(name="psP", bufs=1, space="PSUM") as psP,
        ):
            # --- weights + x in one tile; DMA0 carries consts+piece0 ---
            xall = cpool.tile([128, WTOT], f32r)
            xcs = cpool.tile([128, C_TOT], f32)
            wg_t = xall[:, W_WG:W_WG + 66]
            wft_t = xall[:, W_WF:W_WF + 64]
            wqt_t = xall[:, W_WQ:W_WQ + 64]                     # [128, 64]
            idr_t = xall[:, W_IDR:W_IDR + 128]                  # [128,128]
            wot_t = xcs[0:33, C_WO:C_WO + 128].bitcast(bf16)    # [33, 256]
            idm_t = xcs[:, C_ID:C_ID + 64].bitcast(bf16)        # [128, 128]
            # piece0 via SP HWDGE; small consts + xc via ACT HWDGE;
            # remaining pieces on SP
            nc.sync.dma_start(xall[:, 0:W_WF], xw[:, 0:W_WF])
            nc.scalar.dma_start(xall[:, W_WF:W_CON], xw[:, W_WF:W_CON])
            nc.scalar.dma_start(xcs[:, :], xc[:, :])
            for gp in range(1, 8):
                s0 = W_CON + (gp - 1) * 1024
                nc.sync.dma_start(xall[:, s0:s0 + 1024], xw[:, s0:s0 + 1024])

            def xv(c, col, w):
                # x chunk c (c in 0..1), columns [col, col+w) piece-major
                gp = col // G
                assert col % G + w <= G
                base = W_P0 if gp == 0 else W_CON + (gp - 1) * 1024
                return xall[:, base + c * G + col % G:
                            base + c * G + col % G + w]

            # --- activation buffers ---
            f_t = []
            for gi in range(NIG):
                ft = cpool.tile([33, G], fp8, name=f"f{gi}")
                f_t.append(ft)
                nc.vector.memset(ft[32:33, :], 1.0)
            g_aug = cpool.tile([33, N], fp8)      # rows: g(32), gbv(1)
            hpt = cpool.tile([128, NJT * 33], bf16)
            hpt_v = hpt[:].rearrange("p (t w) -> p t w", w=33)
            # num^T/den accumulators for both ig parities. NOTE: nothing
            # else may live in this bank: any other matmul's start=True
            # would mark the whole 2KB zero-region pending and corrupt
            # the accumulation.
            po_all = psP.tile([128, 2 * 4 * 34], f32, name="po")

            # --- g conv: [33, w] per group (col0/w used to split grp 0
            # during startup so mm1(0) can begin after the first half) ---
            def emit_g_conv(grp, col0=0, w=G):
                cps = psC.tile([33, w], f32, name="cv", tag="cv")
                for c in range(2):
                    nc.tensor.matmul(
                        cps[:, :],
                        wg_t[:, c * 33:(c + 1) * 33],
                        xv(c, grp * G + col0, w),
                        start=(c == 0), stop=(c == 1))
                nc.vector.tensor_copy(
                    g_aug[:, grp * G + col0:grp * G + col0 + w], cps[:, :])

            # --- h^T conv: 4 j-tiles per group, out [j, 32] directly ---
            def emit_h_conv(grp):
                hps = psC.tile([128, 4 * 32], f32, name="cv", tag="cv")
                for k in range(4):
                    jt = 4 * grp + k
                    for c in range(2):
                        nc.tensor.matmul(
                            hps[:, k * 32:(k + 1) * 32],
                            xv(c, jt * 128, 128),
                            wqt_t[:, c * 32:(c + 1) * 32],
                            start=(c == 0), stop=(c == 1))
                nc.vector.tensor_copy(
                    hpt_v[:, 4 * grp:4 * grp + 4, 0:32],
                    hps[:, :].rearrange("p (t w) -> p t w", w=32))
                nc.vector.memset(hpt_v[:, 4 * grp:4 * grp + 4, 32:33], 1.0)

            # --- f conv (own query half): f = Wv @ xq, fp8 out ---
            def emit_f_conv(fg):
                cps = psC.tile([32, G], f32, name="cv", tag="cv")
                for c in range(2):
                    nc.tensor.matmul(
                        cps[:, :],
                        wft_t[:, c * 32:(c + 1) * 32],
                        xv(c, fg * G, G),
                        start=(c == 0), stop=(c == 1))
                nc.vector.tensor_copy(f_t[fg][0:32, :], cps[:, :])

            # --- main attention loop ---
            stages = [(g, si) for g in range(NIG) for si in range(NST)]
            NS = len(stages)

            po_t = {}
            att_t = {}
            sps_t = {}
            eb_t = {}
            rd_t = {}
            op_t = {}
            rt_t = {}

            def emit_mm1(idx):
                g, si = stages[idx]
                nt = SUPERS[si]
                sps = psS.tile([128, SUP * G], f32, name="s")
                sps_t[idx] = sps
                fr = f_t[g][:, :].unsqueeze(1).broadcast_to([33, 2, G])
                for t in range(nt):
                    jt = JT0[si] + t
                    gl = (g_aug[:, jt * 128:(jt + 1) * 128]
                          .unsqueeze(1).broadcast_to([33, 2, 128]))
                    nc.tensor.matmul(
                        sps[:, t * G:(t + 1) * G], gl, fr,
                        start=True, stop=True, perf_mode=DR)

            def emit_exp(idx):
                eng = sched[idx]
                g, si = stages[idx]
                w = SUPERS[si] * G
                eb = epool.tile([128, SUP * G], bf16, name="eb")
                eb_t[idx] = eb
                sps = sps_t.pop(idx)
                if eng == "A":
                    nc.scalar.activation(eb[:, 0:w], sps[:, 0:w], Exp,
                                         scale=0.5)
                else:
                    e = nc.vector if eng == "D" else nc.gpsimd
                    e.tensor_scalar(eb[:, 0:w].bitcast(i16), sps[:, 0:w],
                                    A_SCH, B_SCH, mult, add)
                if DBG and idx == 0:
                    t = rpool.tile([128, SUP * G], f32, name="dbe")
                    nc.vector.tensor_copy(t[:, :], eb[:, :])
                    nc.sync.dma_start(dbg_eb, t[:, :])

            def emit_mm2(idx):
                g, si = stages[idx]
                eb = eb_t.pop(idx)
                if si == 0:
                    po_t[g] = po_all[:, (g % 2) * 136:(g % 2) * 136 + 136]
                for t in range(SUPERS[si]):
                    jt = JT0[si] + t
                    for c in range(4):
                        nc.tensor.matmul(
                            po_t[g][:, c * 34:c * 34 + 33],
                            eb[:, (t * 4 + c) * 128:(t * 4 + c + 1) * 128],
                            hpt_v[:, jt],
                            start=(jt == 0), stop=(jt == NJT - 1))

            # --- tail: scale -> transpose -> out conv (+x via PE) ---
            def emit_tail_scale(g):
                # rden for all 4 chunks in one strided reciprocal, then
                # att^T[i, 0:33] bf16 = po * rden (row 32 -> ~1.0, which
                # doubles as the out-conv bias-ones row)
                rd = spool.tile([128, 4], f32, name="rd")
                pv = po_t[g][:, :].rearrange("p (c w) -> p c w", w=34)
                with nc.allow_low_precision(reason="softmax denom"):
                    nc.vector.reciprocal(rd[:, :], pv[:, :, 32])
                asc = spool.tile([128, 4 * 34], bf16, name="asc")
                nc.vector.tensor_tensor(
                    asc[:, :].rearrange("p (c w) -> p c w", w=34),
                    pv[:, :, :],
                    rd[:, :].unsqueeze(2).broadcast_to([128, 4, 34]),
                    mult)
                rd_t[g] = (rd, asc)
                if DBG and g == 0:
                    t = rpool.tile([128, 136], f32, name="dbp")
                    nc.vector.tensor_copy(t[:, :], po_t[g][:, :])
                    nc.sync.dma_start(dbg_po, t[:, :])
                    t2 = rpool.tile([128, 136], f32, name="dba")
                    nc.vector.tensor_copy(t2[:, :], asc[:, :])
                    nc.sync.dma_start(dbg_asc, t2[:, :])

            def emit_tail_tps(g):
                _, asc = rd_t[g]
                atp = psC.tile([33, 512], bf16, name="cv", tag="cv")
                for c in range(4):
                    nc.tensor.transpose(
                        atp[:, c * 128:(c + 1) * 128],
                        asc[:, c * 34:c * 34 + 33], idm_t)
                att = spool.tile([33, 512], bf16, name="att")
                nc.vector.tensor_copy(att[:, :], atp[:, :])
                att_t[g] = att
                if DBG and g == 0:
                    t = rpool.tile([33, 512], f32, name="dbt")
                    nc.vector.tensor_copy(t[:, :], att[:, :])
                    nc.sync.dma_start(dbg_att, t[:, :])

            KN_RESID = os.environ.get("KN_RESID", "pe")

            def emit_tail_conv(g, cc):
                att = att_t[g]
                if cc == 0:
                    op = psS.tile([128, 1024], f32, name="s", tag="s")
                    op_t[g] = op
                op = op_t[g]
                nc.tensor.matmul(
                    op[:, cc * G:(cc + 1) * G], idr_t,
                    xv(cc, g * G, G),
                    start=True, stop=False, skip_group_check=True)
                nc.tensor.matmul(
                    op[:, cc * G:(cc + 1) * G],
                    wot_t[:, cc * 128:(cc + 1) * 128],
                    att[:, :], start=False, stop=True,
                    skip_group_check=True)

            def emit_tail_out(g, cc):
                op = op_t[g]
                if cc == 0:
                    rt_t[g] = rpool.tile([128, 1024], f32, name="rt")
                rt = rt_t[g]
                out_v = res.rearrange("(c p) (gg n) -> p gg c n",
                                      c=2, n=G)[:, g]
                if DBG and g == 0 and cc == 0:
                    t = rpool.tile([128, 1024], f32, name="dbo")
                    nc.vector.tensor_copy(t[:, :], op[:, :])
                    nc.sync.dma_start(dbg_op, t[:, :])
                    t2 = rpool.tile([33, 256], f32, name="dbw")
                    nc.vector.tensor_copy(t2[:, :], wot_t[:, :])
                    nc.sync.dma_start(dbg_wot, t2[:, :])
                nc.vector.tensor_copy(rt[:, cc * G:(cc + 1) * G],
                                      op[:, cc * G:(cc + 1) * G])
                nc.sync.dma_start(out_v[:, cc], rt[:, cc * G:(cc + 1) * G])
                if cc == 1:
                    po_t.pop(g)
                    rd_t.pop(g)
                    att_t.pop(g)
                    op_t.pop(g)
                    rt_t.pop(g)

            # --- pipeline ---
            KN_CAH = int(os.environ.get("KN_CAH", "6"))
            KN_FSI = int(os.environ.get("KN_FSI", str((NST * 11) // 16)))
            KN_TDL = int(os.environ.get("KN_TDL", "1"))
            convs_left = list(range(1, 8))
            f_left = list(range(1, NIG))
            pending = []   # (g, tail_step, stage_done)

            TAIL = [lambda g: emit_tail_scale(g),
                    lambda g: emit_tail_tps(g),
                    lambda g: emit_tail_conv(g, 0),
                    lambda g: emit_tail_conv(g, 1),
                    lambda g: emit_tail_out(g, 0),
                    lambda g: emit_tail_out(g, 1)]
            NTS = len(TAIL)

            # mm2 runs one stage behind (KN_MM2D=1): by the time mm2(k-1)
            # reaches the PE queue head its exp has long finished, so a
            # slow exp never stalls the next stage's mm1 behind it.
            KN_MM2D = int(os.environ.get("KN_MM2D", "1"))
            emit_g_conv(0)
            emit_f_conv(0)
            emit_mm1(0)
            emit_h_conv(0)
            for k in range(NS):
                g, si = stages[k]
                emit_exp(k)
                if k + 1 < NS:
                    emit_mm1(k + 1)
                if k >= KN_MM2D and os.environ.get("KN_M2P", "late") == "early":
                    emit_mm2(k - KN_MM2D)
                if g == 0:
                    need = min((JT0[si] + SUPERS[si] + KN_CAH) // 4, 7)
                    while convs_left and convs_left[0] <= need:
                        gp = convs_left.pop(0)
                        emit_g_conv(gp)
                        emit_h_conv(gp)
                if f_left and si >= KN_FSI and f_left[0] <= g + 1:
                    emit_f_conv(f_left.pop(0))
                if pending and k >= pending[0][2] + KN_TDL:
                    gg, step, _ = pending[0]
                    TAIL[step](gg)
                    if step == NTS - 1:
                        pending.pop(0)
                    else:
                        pending[0] = (gg, step + 1, pending[0][2])
                if k >= KN_MM2D and os.environ.get("KN_M2P", "late") != "early":
                    emit_mm2(k - KN_MM2D)
                if si == NST - 1:
                    pending.append((g, 0, k + KN_MM2D))
            for k in range(max(0, NS - KN_MM2D), NS):
                emit_mm2(k)
            while convs_left:
                gp = convs_left.pop(0)
                emit_g_conv(gp)
                emit_h_conv(gp)
            while f_left:
                emit_f_conv(f_left.pop(0))
            def emit_tail_drain(g):
                # last tail: per-chunk pipeline across DVE/PE so the
                # serial drain chain is as short as possible
                rd = spool.tile([128, 4], f32, name="rd")
                pv = po_t[g][:, :].rearrange("p (c w) -> p c w", w=34)
                asc = spool.tile([128, 4 * 34], bf16, name="asc")
                ascv = asc[:, :].rearrange("p (c w) -> p c w", w=34)
                atp = psC.tile([33, 512], bf16, name="cv", tag="cv")
                att = spool.tile([33, 512], bf16, name="att")
                op = psS.tile([128, 1024], f32, name="s", tag="s")
                rt = rpool.tile([128, 1024], f32, name="rt")
                with nc.allow_low_precision(reason="softmax denom"):
                    nc.vector.reciprocal(rd[:, :], pv[:, :, 32])
                for cc in range(2):
                    nc.tensor.matmul(
                        op[:, cc * G:(cc + 1) * G], idr_t, xv(cc, g * G, G),
                        start=True, stop=False, skip_group_check=True)
                for c in range(4):
                    nc.vector.tensor_tensor(
                        ascv[:, c], pv[:, c],
                        rd[:, c:c + 1].broadcast_to([128, 34]), mult)
                    nc.tensor.transpose(
                        atp[:, c * 128:(c + 1) * 128],
                        asc[:, c * 34:c * 34 + 33], idm_t)
                    nc.vector.tensor_copy(att[:, c * 128:(c + 1) * 128],
                                          atp[:, c * 128:(c + 1) * 128])
                    for cc in range(2):
                        nc.tensor.matmul(
                            op[:, cc * G + c * 128:cc * G + (c + 1) * 128],
                            wot_t[:, cc * 128:(cc + 1) * 128],
                            att[:, c * 128:(c + 1) * 128],
                            start=False, stop=(c == 3),
                            skip_group_check=True)
                out_v = res.rearrange("(c p) (gg n) -> p gg c n",
                                      c=2, n=G)[:, g]
                nc.vector.tensor_copy(rt[:, 0:G], op[:, 0:G])
                nc.sync.dma_start(out_v[:, 0], rt[:, 0:G])
                nc.scalar.copy(rt[:, G:2 * G], op[:, G:2 * G])
                nc.sync.dma_start(out_v[:, 1], rt[:, G:2 * G])
                po_t.pop(g)

            while pending:
                gg, step, _ = pending.pop(0)
                if step == 0 and os.environ.get("KN_DRAIN", "steps") == "pipe":
                    emit_tail_drain(gg)
                else:
                    for st in range(step, NTS):
                        TAIL[st](gg)
            if DBG:
                t = rpool.tile([33, N], f32, name="dbg")
                nc.vector.tensor_copy(t[:, :], g_aug[:, :])
                nc.sync.dma_start(dbg_g, t[:, :])
                t = rpool.tile([33, G], f32, name="dbf")
                nc.vector.tensor_copy(t[:, :], f_t[0][:, :])
                nc.sync.dma_start(dbg_f, t[:, :])
                t = rpool.tile([128, NJT * 33], f32, name="dbh")
                nc.vector.tensor_copy(t[:, :], hpt[:, :])
                nc.sync.dma_start(dbg_h, t[:, :])

    nc.compile()
    return nc


def _host_prep(Wv, bv, Wk, bk, Wq, bq, Wo, bo, gamma):
    import ml_dtypes
    bfd = ml_dtypes.bfloat16
    gam = float(np.asarray(gamma).reshape(-1)[0])

    # g conv lhsT: rows [Wk(32); bv@Wk(1)]
    w_g = np.zeros((33, 256), np.float32)
    w_g[0:32] = Wk
    w_g[32] = bv @ Wk
    wg = np.zeros((128, 66), np.float32)
    for c in range(2):
        wg[:, c * 33:(c + 1) * 33] = w_g.T[c * 128:(c + 1) * 128, :]

    wft = np.zeros((128, 64), np.float32)
    for c in range(2):
        wft[:, c * 32:(c + 1) * 32] = Wv.T[c * 128:(c + 1) * 128, :]

    # out conv lhsT rows k: k<32 -> gamma*Wo^T, k==32 -> bias row (bf16)
    bof = gam * (Wo @ bq + bo)                                  # [256]
    wot = np.zeros((33, 256), np.float32)
    for c in range(2):
        wot[0:32, c * 128:(c + 1) * 128] = gam * Wo[c * 128:(c + 1) * 128, :].T
        wot[32, c * 128:(c + 1) * 128] = bof[c * 128:(c + 1) * 128]

    wqt = np.zeros((128, 64), np.float32)   # bf16 [128, 2x32]
    for c in range(2):
        wqt[:, c * 32:(c + 1) * 32] = Wq.T[c * 128:(c + 1) * 128, :]

    def pack_bf16(a):
        u = a.astype(bfd).view(np.uint16).astype(np.uint32)
        return (u[:, 0::2] | (u[:, 1::2] << 16)).view(np.float32)

    wpk = np.zeros((128, W_CON), np.float32)
    wpk[:, W_WG:W_WG + 66] = wg
    wpk[:, W_WF:W_WF + 64] = wft
    wpk[:, W_WQ:W_WQ + 64] = wqt
    wpk[:, W_IDR:W_IDR + 128] = np.eye(128, dtype=np.float32)
    cpk = np.zeros((128, C_TOT), np.float32)
    cpk[0:33, C_WO:C_WO + 128] = pack_bf16(wot)
    cpk[:, C_ID:C_ID + 64] = pack_bf16(np.eye(128, dtype=np.float32))
    return wpk, cpk


def kernel(**inputs):
    from concourse.bass_utils import run_bass_kernel_spmd

    x = np.asarray(inputs["x"], np.float32)
    consts, cpk = _host_prep(
        np.asarray(inputs["Wv"], np.float32),
        np.asarray(inputs["bv"], np.float32),
        np.asarray(inputs["Wk"], np.float32),
        np.asarray(inputs["bk"], np.float32),
        np.asarray(inputs["Wq"], np.float32),
        np.asarray(inputs["bq"], np.float32),
        np.asarray(inputs["Wo"], np.float32),
        np.asarray(inputs["bo"], np.float32),
        np.asarray(inputs["gamma"], np.float32),
    )

    if "nc" not in _CACHE:
        _CACHE["nc"] = build_program()
    nc = _CACHE["nc"]

    in_maps = []
    for core in range(NCORES):
        b, ih = core // 2, core % 2
        xrot = np.roll(x[b], -ih * NH, axis=1)
        xp = (xrot.reshape(2, 128, 8, 512)
              .transpose(1, 2, 0, 3).reshape(128, 8, 1024))
        xwb = np.empty((128, WTOT), np.float32)
        xwb[:, 0:W_CON] = consts
        xwb[:, W_P0:W_P0 + 1024] = xp[:, 0]
        xwb[:, W_CON:] = xp[:, 1:].reshape(128, 7 * 1024)
        in_maps.append({"xw": xwb, "xc": cpk})

    r = run_bass_kernel_spmd(nc, in_maps, core_ids=list(range(NCORES)),
                             trace=False)
    out = np.empty((B, C, N), np.float32)
    for core in range(NCORES):
        b, ih = core // 2, core % 2
        out[b][:, ih * NH:(ih + 1) * NH] = r.results[core]["res"]
    return out


if __name__ == "__main__":
    nc = build_program()
    print("program built ok")


# revision 101
# speedup vs baseline: 1.0350x; 1.0350x over previous
"""Trainium2 Bass kernel for nn_AttentionLayer (B=4, C=256, N=4096, CR=32).

Sharding: 8 cores = (batch b in 0..3) x (query-half ih in 0..1).
Each core receives x[b] rotated so its own query half sits at columns
0..2047 (softmax is invariant to key order, so the rotation is exact);
it computes out[b][:, ih*2048:(ih+1)*2048] and the host reassembles.

Per-core algorithm:
  - g conv (keys + gbv bias-correction row) in f32r: [33,C] @ x -> g
  - h^T conv: lhsT = x chunk (stationary), rhs = Wq^T bf16 (moving,
    ap=32) -> h^T j-tiles directly in [j, 32] layout (no PE transpose)
  - f conv (queries, own half) in f32r; f/g stored as fp8e4m3 in SBUF
  - scores via fp8 DoubleRow matmul with a broadcast (stride-0) slot
    dim on both operands: psum = 2*(g_aug^T f_aug), 0.5 cycles/row.
    The 2x is undone inside exp (scale=0.5).
  - exp split across three engines: ACT native Exp; DVE/Pool compute
    Schraudolph bits = round(s*64/ln2 + B) written as int16 == bf16.
  - mm2 swapped: lhsT = eb (stationary bf16), rhs = hpt [j,33] bf16
    (moving, ap=33) accumulating num^T/den in [i, 33] psum chunks.
  - tail per i-chunk: rden = 1/den (per-partition), att^T = po*rden
    (bf16; row 32 becomes den*rden ~= 1 and doubles as the out-conv
    bias-ones row), PE transpose (bf16 identity), out conv
    (gamma*Wo^T | bias row), residual add fused into PSUM->SBUF copy.
"""

import os
import numpy as np

B, C, N = 4, 256, 4096
CR = 32
NH = N // 2          # queries per core
G = 512              # i-group width
NCORES = 8

NJT = N // 128       # 32 j-tiles
NIG = NH // G        # 4 i-groups
SUP = int(os.environ.get("KN_SUP", "2"))   # max j-tiles per stage
# per-i-group supers list (sums to NJT)
SUPERS = [SUP] * (NJT // SUP) + ([NJT % SUP] if NJT % SUP else [])
NST = len(SUPERS)    # stages per i-group
JT0 = [sum(SUPERS[:i]) for i in range(NST)]

# xw (f32r) layout: true-f32r data, DMA TF32 rounding is acceptable
W_WG = 0             # g conv lhsT   [128, 66]  (2 chunks x 33)
W_P0 = 66            # x piece 0     [128, 1024]
W_WF = 1090          # f conv lhsT   [128, 64]  (2 chunks x 32)
W_WQ = 1154          # wqt f32r      [128, 2x32]
W_IDR = 1218         # idr f32r identity [128, 128]
W_CON = 1346         # end of consts
WTOT = W_CON + 7 * 1024
# xc (f32, bit-exact DMA) layout: bit-packed bf16 constants
C_WO = 0             # wotb bf16 [33, 256] packed as u32 [33, 128]
C_ID = 128           # idm128 bf16 [128, 128] packed as u32 [128, 64]
C_TOT = 192

_CACHE = {}


def build_program():
    import concourse.bacc as bacc
    import concourse.mybir as mybir
    from concourse.tile import TileContext

    dt = mybir.dt
    f32 = dt.float32
    f32r = dt.float32r
    bf16 = dt.bfloat16
    fp8 = dt.float8e4
    i16 = dt.int16
    Exp = mybir.ActivationFunctionType.Exp
    add = mybir.AluOpType.add
    mult = mybir.AluOpType.mult
    DR = mybir.MatmulPerfMode.DoubleRow

    A_SCH = 64.0 / np.log(2.0)          # schraudolph slope on 2s input
    B_SCH = 127.0 * 128.0 - 7.0 + 0.5   # bias incl +0.5 for truncation

    nc = bacc.Bacc("TRN2", target_bir_lowering=False, debug=False,
                   num_devices=NCORES)

    # xw is f32r (DMA rounds to TF32 - fine for x and real weights); the
    # bit-packed bf16 constants ride in xc as plain f32 (bit-exact DMA).
    xw = nc.dram_tensor("xw", [128, WTOT], f32r, kind="ExternalInput").ap()
    xc = nc.dram_tensor("xc", [128, C_TOT], f32, kind="ExternalInput").ap()
    res = nc.dram_tensor("res", [C, NH], f32, kind="ExternalOutput").ap()
    DBG = os.environ.get("KN_DEBUG", "") == "1"
    if DBG:
        dbg_g = nc.dram_tensor("dbg_g", [33, N], f32, kind="ExternalOutput").ap()
        dbg_f = nc.dram_tensor("dbg_f", [33, G], f32, kind="ExternalOutput").ap()
        dbg_h = nc.dram_tensor("dbg_h", [128, NJT * 33], f32, kind="ExternalOutput").ap()
        dbg_eb = nc.dram_tensor("dbg_eb", [128, SUP * G], f32, kind="ExternalOutput").ap()
        dbg_po = nc.dram_tensor("dbg_po", [128, 136], f32, kind="ExternalOutput").ap()
        dbg_asc = nc.dram_tensor("dbg_asc", [128, 136], f32, kind="ExternalOutput").ap()
        dbg_att = nc.dram_tensor("dbg_att", [33, 512], f32, kind="ExternalOutput").ap()
        dbg_wot = nc.dram_tensor("dbg_wot", [33, 256], f32, kind="ExternalOutput").ap()
        dbg_op = nc.dram_tensor("dbg_op", [128, 1024], f32, kind="ExternalOutput").ap()

    # exp engine schedule per stage: A=ACT native exp, D=DVE schraudolph.
    # ig0 keeps DVE mostly free for conv copies; later igs alternate more.
    sched = os.environ.get(
        "KN_EXP", "A" * NST + "AADAADAADAA"[:NST] * 3 if NST == 11
        else "A" * 16 + "AADADAADADAADADA" * 3)
    assert len(sched) >= NIG * NST

    with TileContext(nc) as tc:
        with (
            tc.tile_pool(name="const", bufs=1) as cpool,
            tc.tile_pool(name="eb", bufs=6) as epool,
            tc.tile_pool(name="small", bufs=2) as spool,
            tc.tile_pool(name="resp", bufs=2) as rpool,
            tc.tile_pool(name="psS", bufs=(3 if SUP == 2 else 2), space="PSUM") as psS,
            tc.tile_pool(name="psC", bufs=1, space="PSUM") as psC,
            tc.tile_pool(name="psP", bufs=1, space="PSUM") as psP,
        ):
            # --- weights + x in one tile; DMA0 carries consts+piece0 ---
            xall = cpool.tile([128, WTOT], f32r)
            xcs = cpool.tile([128, C_TOT], f32)
            wg_t = xall[:, W_WG:W_WG + 66]
            wft_t = xall[:, W_WF:W_WF + 64]
            wqt_t = xall[:, W_WQ:W_WQ + 64]                     # [128, 64]
            idr_t = xall[:, W_IDR:W_IDR + 128]                  # [128,128]
            wot_t = xcs[0:33, C_WO:C_WO + 128].bitcast(bf16)    # [33, 256]
            idm_t = xcs[:, C_ID:C_ID + 64].bitcast(bf16)        # [128, 128]
            # piece0 via SP HWDGE; small consts + xc via ACT HWDGE;
            # remaining pieces on SP
            nc.sync.dma_start(xall[:, 0:W_WF], xw[:, 0:W_WF])
            nc.scalar.dma_start(xall[:, W_WF:W_CON], xw[:, W_WF:W_CON])
            nc.scalar.dma_start(xcs[:, :], xc[:, :])
            for gp in range(1, 8):
                s0 = W_CON + (gp - 1) * 1024
                nc.sync.dma_start(xall[:, s0:s0 + 1024], xw[:, s0:s0 + 1024])

            def xv(c, col, w):
                # x chunk c (c in 0..1), columns [col, col+w) piece-major
                gp = col // G
                assert col % G + w <= G
                base = W_P0 if gp == 0 else W_CON + (gp - 1) * 1024
                return xall[:, base + c * G + col % G:
                            base + c * G + col % G + w]

            # --- activation buffers ---
            f_t = []
            for gi in range(NIG):
                ft = cpool.tile([33, G], fp8, name=f"f{gi}")
                f_t.append(ft)
                nc.vector.memset(ft[32:33, :], 1.0)
            g_aug = cpool.tile([33, N], fp8)      # rows: g(32), gbv(1)
            hpt = cpool.tile([128, NJT * 33], bf16)
            hpt_v = hpt[:].rearrange("p (t w) -> p t w", w=33)
            # num^T/den accumulators for both ig parities. NOTE: nothing
            # else may live in this bank: any other matmul's start=True
            # would mark the whole 2KB zero-region pending and corrupt
            # the accumulation.
            po_all = psP.tile([128, 2 * 4 * 34], f32, name="po")

            # --- g conv: [33, w] per group (col0/w used to split grp 0
            # during startup so mm1(0) can begin after the first half) ---
            def emit_g_conv(grp, col0=0, w=G):
                cps = psS.tile([33, w], f32, name="s", tag="s") if os.environ.get("KN_GS","0")=="1" else psC.tile([33, w], f32, name="cv", tag="cv")
                for c in range(2):
                    nc.tensor.matmul(
                        cps[:, :],
                        wg_t[:, c * 33:(c + 1) * 33],
                        xv(c, grp * G + col0, w),
                        start=(c == 0), stop=(c == 1))
                nc.vector.tensor_copy(
                    g_aug[:, grp * G + col0:grp * G + col0 + w], cps[:, :])

            # --- h^T conv: 4 j-tiles per group, out [j, 32] directly ---
            def emit_h_conv(grp):
                hps = psS.tile([128, 4 * 32], f32, name="s", tag="s")
                for k in range(4):
                    jt = 4 * grp + k
                    for c in range(2):
                        nc.tensor.matmul(
                            hps[:, k * 32:(k + 1) * 32],
                            xv(c, jt * 128, 128),
                            wqt_t[:, c * 32:(c + 1) * 32],
                            start=(c == 0), stop=(c == 1))
                nc.vector.tensor_copy(
                    hpt_v[:, 4 * grp:4 * grp + 4, 0:32],
                    hps[:, :].rearrange("p (t w) -> p t w", w=32))
                nc.vector.memset(hpt_v[:, 4 * grp:4 * grp + 4, 32:33], 1.0)

            # --- f conv (own query half): f = Wv @ xq, fp8 out ---
            def emit_f_conv(fg):
                cps = psS.tile([32, G], f32, name="s", tag="s")
                for c in range(2):
                    nc.tensor.matmul(
                        cps[:, :],
                        wft_t[:, c * 32:(c + 1) * 32],
                        xv(c, fg * G, G),
                        start=(c == 0), stop=(c == 1))
                nc.vector.tensor_copy(f_t[fg][0:32, :], cps[:, :])

            # --- main attention loop ---
            stages = [(g, si) for g in range(NIG) for si in range(NST)]
            NS = len(stages)

            po_t = {}
            att_t = {}
            sps_t = {}
            eb_t = {}
            rd_t = {}
            op_t = {}
            rt_t = {}

            def emit_mm1(idx):
                g, si = stages[idx]
                nt = SUPERS[si]
                sps = psS.tile([128, SUP * G], f32, name="s")
                sps_t[idx] = sps
                fr = f_t[g][:, :].unsqueeze(1).broadcast_to([33, 2, G])
                for t in range(nt):
                    jt = JT0[si] + t
                    gl = (g_aug[:, jt * 128:(jt + 1) * 128]
                          .unsqueeze(1).broadcast_to([33, 2, 128]))
                    nc.tensor.matmul(
                        sps[:, t * G:(t + 1) * G], gl, fr,
                        start=True, stop=True, perf_mode=DR)

            def emit_exp(idx):
                eng = sched[idx]
                g, si = stages[idx]
                w = SUPERS[si] * G
                eb = epool.tile([128, SUP * G], bf16, name="eb")
                eb_t[idx] = eb
                sps = sps_t.pop(idx)
                if eng == "A":
                    nc.scalar.activation(eb[:, 0:w], sps[:, 0:w], Exp,
                                         scale=0.5)
                else:
                    e = nc.vector if eng == "D" else nc.gpsimd
                    e.tensor_scalar(eb[:, 0:w].bitcast(i16), sps[:, 0:w],
                                    A_SCH, B_SCH, mult, add)
                if DBG and idx == 0:
                    t = rpool.tile([128, SUP * G], f32, name="dbe")
                    nc.vector.tensor_copy(t[:, :], eb[:, :])
                    nc.sync.dma_start(dbg_eb, t[:, :])

            def emit_mm2(idx):
                g, si = stages[idx]
                eb = eb_t.pop(idx)
                if si == 0:
                    po_t[g] = po_all[:, (g % 2) * 136:(g % 2) * 136 + 136]
                for t in range(SUPERS[si]):
                    jt = JT0[si] + t
                    for c in range(4):
                        nc.tensor.matmul(
                            po_t[g][:, c * 34:c * 34 + 33],
                            eb[:, (t * 4 + c) * 128:(t * 4 + c + 1) * 128],
                            hpt_v[:, jt],
                            start=(jt == 0), stop=(jt == NJT - 1))

            # --- tail: scale -> transpose -> out conv (+x via PE) ---
            def emit_tail_scale(g):
                # rden for all 4 chunks in one strided reciprocal, then
                # att^T[i, 0:33] bf16 = po * rden (row 32 -> ~1.0, which
                # doubles as the out-conv bias-ones row)
                rd = spool.tile([128, 4], f32, name="rd")
                pv = po_t[g][:, :].rearrange("p (c w) -> p c w", w=34)
                with nc.allow_low_precision(reason="softmax denom"):
                    nc.vector.reciprocal(rd[:, :], pv[:, :, 32])
                asc = spool.tile([128, 4 * 34], bf16, name="asc")
                nc.vector.tensor_tensor(
                    asc[:, :].rearrange("p (c w) -> p c w", w=34),
                    pv[:, :, :],
                    rd[:, :].unsqueeze(2).broadcast_to([128, 4, 34]),
                    mult)
                rd_t[g] = (rd, asc)
                if DBG and g == 0:
                    t = rpool.tile([128, 136], f32, name="dbp")
                    nc.vector.tensor_copy(t[:, :], po_t[g][:, :])
                    nc.sync.dma_start(dbg_po, t[:, :])
                    t2 = rpool.tile([128, 136], f32, name="dba")
                    nc.vector.tensor_copy(t2[:, :], asc[:, :])
                    nc.sync.dma_start(dbg_asc, t2[:, :])

            def emit_tail_tps(g):
                _, asc = rd_t[g]
                atp = psC.tile([33, 512], bf16, name="cv", tag="cv")
                for c in range(4):
                    nc.tensor.transpose(
                        atp[:, c * 128:(c + 1) * 128],
                        asc[:, c * 34:c * 34 + 33], idm_t)
                att = spool.tile([33, 512], bf16, name="att")
                if os.environ.get("KN_ATTE", "D") == "A":
                    nc.scalar.copy(att[:, :], atp[:, :])
                else:
                    nc.vector.tensor_copy(att[:, :], atp[:, :])
                att_t[g] = att
                if DBG and g == 0:
                    t = rpool.tile([33, 512], f32, name="dbt")
                    nc.vector.tensor_copy(t[:, :], att[:, :])
                    nc.sync.dma_start(dbg_att, t[:, :])

            KN_RESID = os.environ.get("KN_RESID", "pe")

            def emit_tail_conv(g, cc):
                att = att_t[g]
                if cc == 0:
                    op = psS.tile([128, 1024], f32, name="s", tag="s")
                    op_t[g] = op
                op = op_t[g]
                nc.tensor.matmul(
                    op[:, cc * G:(cc + 1) * G], idr_t,
                    xv(cc, g * G, G),
                    start=True, stop=False, skip_group_check=True)
                nc.tensor.matmul(
                    op[:, cc * G:(cc + 1) * G],
                    wot_t[:, cc * 128:(cc + 1) * 128],
                    att[:, :], start=False, stop=True,
                    skip_group_check=True)

            def emit_tail_out(g, cc):
                op = op_t[g]
                if cc == 0:
                    rt_t[g] = rpool.tile([128, 1024], f32, name="rt")
                rt = rt_t[g]
                out_v = res.rearrange("(c p) (gg n) -> p gg c n",
                                      c=2, n=G)[:, g]
                if DBG and g == 0 and cc == 0:
                    t = rpool.tile([128, 1024], f32, name="dbo")
                    nc.vector.tensor_copy(t[:, :], op[:, :])
                    nc.sync.dma_start(dbg_op, t[:, :])
                    t2 = rpool.tile([33, 256], f32, name="dbw")
                    nc.vector.tensor_copy(t2[:, :], wot_t[:, :])
                    nc.sync.dma_start(dbg_wot, t2[:, :])
                nc.vector.tensor_copy(rt[:, cc * G:(cc + 1) * G],
                                      op[:, cc * G:(cc + 1) * G])
                nc.sync.dma_start(out_v[:, cc], rt[:, cc * G:(cc + 1) * G])
                if cc == 1:
                    po_t.pop(g)
                    rd_t.pop(g)
                    att_t.pop(g)
                    op_t.pop(g)
                    rt_t.pop(g)

            # --- pipeline ---
            KN_CAH = int(os.environ.get("KN_CAH", "6"))
            KN_FSI = int(os.environ.get("KN_FSI", str((NST * 11) // 16)))
            KN_TDL = int(os.environ.get("KN_TDL", "1"))
            convs_left = list(range(1, 8))
            f_left = list(range(1, NIG))
            pending = []   # (g, tail_step, stage_done)

            TAIL = [lambda g: emit_tail_scale(g),
                    lambda g: emit_tail_tps(g),
                    lambda g: emit_tail_conv(g, 0),
                    lambda g: emit_tail_conv(g, 1),
                    lambda g: emit_tail_out(g, 0),
                    lambda g: emit_tail_out(g, 1)]
            NTS = len(TAIL)

            # mm2 runs one stage behind (KN_MM2D=1): by the time mm2(k-1)
            # reaches the PE queue head its exp has long finished, so a
            # slow exp never stalls the next stage's mm1 behind it.
            KN_MM2D = int(os.environ.get("KN_MM2D", "1"))
            emit_g_conv(0)
            emit_f_conv(0)
            emit_mm1(0)
            emit_h_conv(0)
            for k in range(NS):
                g, si = stages[k]
                emit_exp(k)
                if k + 1 < NS:
                    emit_mm1(k + 1)
                if k >= KN_MM2D and os.environ.get("KN_M2P", "late") == "early":
                    emit_mm2(k - KN_MM2D)
                if g == 0:
                    need = min((JT0[si] + SUPERS[si] + KN_CAH) // 4, 7)
                    while convs_left and convs_left[0] <= need:
                        gp = convs_left.pop(0)
                        emit_g_conv(gp)
                        emit_h_conv(gp)
                if f_left and si >= KN_FSI and f_left[0] <= g + 1:
                    emit_f_conv(f_left.pop(0))
                if pending and k >= pending[0][2] + KN_TDL:
                    gg, step, _ = pending[0]
                    TAIL[step](gg)
                    if step == NTS - 1:
                        pending.pop(0)
                    else:
                        pending[0] = (gg, step + 1, pending[0][2])
                if k >= KN_MM2D and os.environ.get("KN_M2P", "late") != "early":
                    emit_mm2(k - KN_MM2D)
                if si == NST - 1:
                    pending.append((g, 0, k + KN_MM2D))
            for k in range(max(0, NS - KN_MM2D), NS):
                emit_mm2(k)
            while convs_left:
                gp = convs_left.pop(0)
                emit_g_conv(gp)
                emit_h_conv(gp)
            while f_left:
                emit_f_conv(f_left.pop(0))
            def emit_tail_drain(g):
                # last tail: per-chunk pipeline across DVE/PE so the
                # serial drain chain is as short as possible
                rd = spool.tile([128, 4], f32, name="rd")
                pv = po_t[g][:, :].rearrange("p (c w) -> p c w", w=34)
                asc = spool.tile([128, 4 * 34], bf16, name="asc")
                ascv = asc[:, :].rearrange("p (c w) -> p c w", w=34)
                atp = psC.tile([33, 512], bf16, name="cv", tag="cv")
                att = spool.tile([33, 512], bf16, name="att")
                op = psS.tile([128, 1024], f32, name="s", tag="s")
                rt = rpool.tile([128, 1024], f32, name="rt")
                with nc.allow_low_precision(reason="softmax denom"):
                    nc.vector.reciprocal(rd[:, :], pv[:, :, 32])
                for cc in range(2):
                    nc.tensor.matmul(
                        op[:, cc * G:(cc + 1) * G], idr_t, xv(cc, g * G, G),
                        start=True, stop=False, skip_group_check=True)
                for c in range(4):
                    nc.vector.tensor_tensor(
                        ascv[:, c], pv[:, c],
                        rd[:, c:c + 1].broadcast_to([128, 34]), mult)
                    nc.tensor.transpose(
                        atp[:, c * 128:(c + 1) * 128],
                        asc[:, c * 34:c * 34 + 33], idm_t)
                    nc.vector.tensor_copy(att[:, c * 128:(c + 1) * 128],
                                          atp[:, c * 128:(c + 1) * 128])
                    for cc in range(2):
                        nc.tensor.matmul(
                            op[:, cc * G + c * 128:cc * G + (c + 1) * 128],
                            wot_t[:, cc * 128:(cc + 1) * 128],
                            att[:, c * 128:(c + 1) * 128],
                            start=False, stop=(c == 3),
                            skip_group_check=True)
                out_v = res.rearrange("(c p) (gg n) -> p gg c n",
                                      c=2, n=G)[:, g]
                nc.vector.tensor_copy(rt[:, 0:G], op[:, 0:G])
                nc.sync.dma_start(out_v[:, 0], rt[:, 0:G])
                nc.scalar.copy(rt[:, G:2 * G], op[:, G:2 * G])
                nc.sync.dma_start(out_v[:, 1], rt[:, G:2 * G])
                po_t.pop(g)

            while pending:
                gg, step, _ = pending.pop(0)
                if step == 0 and os.environ.get("KN_DRAIN", "steps") == "pipe":
                    emit_tail_drain(gg)
                else:
                    for st in range(step, NTS):
                        TAIL[st](gg)
            if DBG:
                t = rpool.tile([33, N], f32, name="dbg")
                nc.vector.tensor_copy(t[:, :], g_aug[:, :])
                nc.sync.dma_start(dbg_g, t[:, :])
                t = rpool.tile([33, G], f32, name="dbf")
                nc.vector.tensor_copy(t[:, :], f_t[0][:, :])
                nc.sync.dma_start(dbg_f, t[:, :])
                t = rpool.tile([128, NJT * 33], f32, name="dbh")
                nc.vector.tensor_copy(t[:, :], hpt[:, :])
                nc.sync.dma_start(dbg_h, t[:, :])

    nc.compile()
    return nc


def _host_prep(Wv, bv, Wk, bk, Wq, bq, Wo, bo, gamma):
    import ml_dtypes
    bfd = ml_dtypes.bfloat16
    gam = float(np.asarray(gamma).reshape(-1)[0])

    # g conv lhsT: rows [Wk(32); bv@Wk(1)]
    w_g = np.zeros((33, 256), np.float32)
    w_g[0:32] = Wk
    w_g[32] = bv @ Wk
    wg = np.zeros((128, 66), np.float32)
    for c in range(2):
        wg[:, c * 33:(c + 1) * 33] = w_g.T[c * 128:(c + 1) * 128, :]

    wft = np.zeros((128, 64), np.float32)
    for c in range(2):
        wft[:, c * 32:(c + 1) * 32] = Wv.T[c * 128:(c + 1) * 128, :]

    # out conv lhsT rows k: k<32 -> gamma*Wo^T, k==32 -> bias row (bf16)
    bof = gam * (Wo @ bq + bo)                                  # [256]
    wot = np.zeros((33, 256), np.float32)
    for c in range(2):
        wot[0:32, c * 128:(c + 1) * 128] = gam * Wo[c * 128:(c + 1) * 128, :].T
        wot[32, c * 128:(c + 1) * 128] = bof[c * 128:(c + 1) * 128]

    wqt = np.zeros((128, 64), np.float32)   # bf16 [128, 2x32]
    for c in range(2):
        wqt[:, c * 32:(c + 1) * 32] = Wq.T[c * 128:(c + 1) * 128, :]

    def pack_bf16(a):
        u = a.astype(bfd).view(np.uint16).astype(np.uint32)
        return (u[:, 0::2] | (u[:, 1::2] << 16)).view(np.float32)

    wpk = np.zeros((128, W_CON), np.float32)
    wpk[:, W_WG:W_WG + 66] = wg
    wpk[:, W_WF:W_WF + 64] = wft
    wpk[:, W_WQ:W_WQ + 64] = wqt
    wpk[:, W_IDR:W_IDR + 128] = np.eye(128, dtype=np.float32)
    cpk = np.zeros((128, C_TOT), np.float32)
    cpk[0:33, C_WO:C_WO + 128] = pack_bf16(wot)
    cpk[:, C_ID:C_ID + 64] = pack_bf16(np.eye(128, dtype=np.float32))
    return wpk, cpk


def kernel(**inputs):
    from concourse.bass_utils import run_bass_kernel_spmd

    x = np.asarray(inputs["x"], np.float32)
    consts, cpk = _host_prep(
        np.asarray(inputs["Wv"], np.float32),
        np.asarray(inputs["bv"], np.float32),
        np.asarray(inputs["Wk"], np.float32),
        np.asarray(inputs["bk"], np.float32),
        np.asarray(inputs["Wq"], np.float32),
        np.asarray(inputs["bq"], np.float32),
        np.asarray(inputs["Wo"], np.float32),
        np.asarray(inputs["bo"], np.float32),
        np.asarray(inputs["gamma"], np.float32),
    )

    if "nc" not in _CACHE:
        _CACHE["nc"] = build_program()
    nc = _CACHE["nc"]

    in_maps = []
    for core in range(NCORES):
        b, ih = core // 2, core % 2
        xrot = np.roll(x[b], -ih * NH, axis=1)
        xp = (xrot.reshape(2, 128, 8, 512)
              .transpose(1, 2, 0, 3).reshape(128, 8, 1024))
        xwb = np.empty((128, WTOT), np.float32)
        xwb[:, 0:W_CON] = consts
        xwb[:, W_P0:W_P0 + 1024] = xp[:, 0]
        xwb[:, W_CON:] = xp[:, 1:].reshape(128, 7 * 1024)
        in_maps.append({"xw": xwb, "xc": cpk})

    r = run_bass_kernel_spmd(nc, in_maps, core_ids=list(range(NCORES)),
                             trace=False)
    out = np.empty((B, C, N), np.float32)
    for core in range(NCORES):
        b, ih = core // 2, core % 2
        out[b][:, ih * NH:(ih + 1) * NH] = r.results[core]["res"]
    return out


if __name__ == "__main__":
    nc = build_program()
    print("program built ok")


# revision 106
# speedup vs baseline: 1.0565x; 1.0207x over previous
"""Trainium2 Bass kernel for nn_AttentionLayer (B=4, C=256, N=4096, CR=32).

Sharding: 8 cores = (batch b in 0..3) x (query-half ih in 0..1).
Each core receives x[b] rotated so its own query half sits at columns
0..2047 (softmax is invariant to key order, so the rotation is exact);
it computes out[b][:, ih*2048:(ih+1)*2048] and the host reassembles.

Per-core algorithm:
  - g conv (keys + gbv bias-correction row) in f32r: [33,C] @ x -> g
  - h^T conv: lhsT = x chunk (stationary), rhs = Wq^T bf16 (moving,
    ap=32) -> h^T j-tiles directly in [j, 32] layout (no PE transpose)
  - f conv (queries, own half) in f32r; f/g stored as fp8e4m3 in SBUF
  - scores via fp8 DoubleRow matmul with a broadcast (stride-0) slot
    dim on both operands: psum = 2*(g_aug^T f_aug), 0.5 cycles/row.
    The 2x is undone inside exp (scale=0.5).
  - exp split across three engines: ACT native Exp; DVE/Pool compute
    Schraudolph bits = round(s*64/ln2 + B) written as int16 == bf16.
  - mm2 swapped: lhsT = eb (stationary bf16), rhs = hpt [j,33] bf16
    (moving, ap=33) accumulating num^T/den in [i, 33] psum chunks.
  - tail per i-chunk: rden = 1/den (per-partition), att^T = po*rden
    (bf16; row 32 becomes den*rden ~= 1 and doubles as the out-conv
    bias-ones row), PE transpose (bf16 identity), out conv
    (gamma*Wo^T | bias row), residual add fused into PSUM->SBUF copy.
"""

import os
import numpy as np

B, C, N = 4, 256, 4096
CR = 32
NH = N // 2          # queries per core
G = 512              # i-group width
NCORES = 8

NJT = N // 128       # 32 j-tiles
NIG = NH // G        # 4 i-groups
SUP = int(os.environ.get("KN_SUP", "2"))   # max j-tiles per stage
# per-i-group supers list (sums to NJT)
SUPERS = [SUP] * (NJT // SUP) + ([NJT % SUP] if NJT % SUP else [])
NST = len(SUPERS)    # stages per i-group
JT0 = [sum(SUPERS[:i]) for i in range(NST)]

# xw (f32r) layout: true-f32r data, DMA TF32 rounding is acceptable
W_WG = 0             # g conv lhsT   [128, 66]  (2 chunks x 33)
W_P0 = 66            # x piece 0     [128, 1024]
W_WF = 1090          # f conv lhsT   [128, 64]  (2 chunks x 32)
W_WQ = 1154          # wqt f32r      [128, 2x32]
W_IDR = 1218         # idr f32r identity [128, 128]
W_CON = 1346         # end of consts
WTOT = W_CON + 7 * 1024
# xc (f32, bit-exact DMA) layout: bit-packed bf16 constants
C_WO = 0             # wotb bf16 [33, 256] packed as u32 [33, 128]
C_ID = 128           # idm128 bf16 [128, 128] packed as u32 [128, 64]
C_TOT = 192

_CACHE = {}


def build_program():
    import concourse.bacc as bacc
    import concourse.mybir as mybir
    from concourse.tile import TileContext

    dt = mybir.dt
    f32 = dt.float32
    f32r = dt.float32r
    bf16 = dt.bfloat16
    fp8 = dt.float8e4
    i16 = dt.int16
    Exp = mybir.ActivationFunctionType.Exp
    add = mybir.AluOpType.add
    mult = mybir.AluOpType.mult
    DR = mybir.MatmulPerfMode.DoubleRow

    A_SCH = 64.0 / np.log(2.0)          # schraudolph slope on 2s input
    B_SCH = 127.0 * 128.0 - 7.0 + 0.5   # bias incl +0.5 for truncation

    nc = bacc.Bacc("TRN2", target_bir_lowering=False, debug=False,
                   num_devices=NCORES)

    # xw is f32r (DMA rounds to TF32 - fine for x and real weights); the
    # bit-packed bf16 constants ride in xc as plain f32 (bit-exact DMA).
    xw = nc.dram_tensor("xw", [128, WTOT], f32r, kind="ExternalInput").ap()
    xc = nc.dram_tensor("xc", [128, C_TOT], f32, kind="ExternalInput").ap()
    res = nc.dram_tensor("res", [C, NH], f32, kind="ExternalOutput").ap()
    DBG = os.environ.get("KN_DEBUG", "") == "1"
    if DBG:
        dbg_g = nc.dram_tensor("dbg_g", [33, N], f32, kind="ExternalOutput").ap()
        dbg_f = nc.dram_tensor("dbg_f", [33, G], f32, kind="ExternalOutput").ap()
        dbg_h = nc.dram_tensor("dbg_h", [128, NJT * 33], f32, kind="ExternalOutput").ap()
        dbg_eb = nc.dram_tensor("dbg_eb", [128, SUP * G], f32, kind="ExternalOutput").ap()
        dbg_po = nc.dram_tensor("dbg_po", [128, 136], f32, kind="ExternalOutput").ap()
        dbg_asc = nc.dram_tensor("dbg_asc", [128, 136], f32, kind="ExternalOutput").ap()
        dbg_att = nc.dram_tensor("dbg_att", [33, 512], f32, kind="ExternalOutput").ap()
        dbg_wot = nc.dram_tensor("dbg_wot", [33, 256], f32, kind="ExternalOutput").ap()
        dbg_op = nc.dram_tensor("dbg_op", [128, 1024], f32, kind="ExternalOutput").ap()

    # exp engine schedule per stage: A=ACT native exp, D=DVE schraudolph.
    # ig0 keeps DVE mostly free for conv copies; later igs alternate more.
    sched = os.environ.get(
        "KN_EXP", "A" * NST + "AADAADAADAA"[:NST] * 3 if NST == 11
        else "A" * 16 + "AADADAADADAADADA" * 3)
    assert len(sched) >= NIG * NST

    with TileContext(nc) as tc:
        with (
            tc.tile_pool(name="const", bufs=1) as cpool,
            tc.tile_pool(name="eb", bufs=6) as epool,
            tc.tile_pool(name="small", bufs=2) as spool,
            tc.tile_pool(name="resp", bufs=2) as rpool,
            tc.tile_pool(name="psS", bufs=(3 if SUP == 2 else 2), space="PSUM") as psS,
            tc.tile_pool(name="psC", bufs=1, space="PSUM") as psC,
            tc.tile_pool(name="psP", bufs=1, space="PSUM") as psP,
        ):
            # --- weights + x in one tile; DMA0 carries consts+piece0 ---
            xall = cpool.tile([128, WTOT], f32r)
            xcs = cpool.tile([128, C_TOT], f32)
            wg_t = xall[:, W_WG:W_WG + 66]
            wft_t = xall[:, W_WF:W_WF + 64]
            wqt_t = xall[:, W_WQ:W_WQ + 64]                     # [128, 64]
            idr_t = xall[:, W_IDR:W_IDR + 128]                  # [128,128]
            wot_t = xcs[0:33, C_WO:C_WO + 128].bitcast(bf16)    # [33, 256]
            idm_t = xcs[:, C_ID:C_ID + 64].bitcast(bf16)        # [128, 128]
            # piece0 via SP HWDGE; small consts + xc via ACT HWDGE;
            # remaining pieces on SP
            nc.sync.dma_start(xall[:, 0:W_P0 + 512], xw[:, 0:W_P0 + 512])
            nc.sync.dma_start(xall[:, W_P0 + 512:W_WF],
                              xw[:, W_P0 + 512:W_WF])
            nc.scalar.dma_start(xall[:, W_WF:W_CON], xw[:, W_WF:W_CON])
            nc.scalar.dma_start(xcs[:, :], xc[:, :])
            for gp in range(1, 8):
                s0 = W_CON + (gp - 1) * 1024
                nc.sync.dma_start(xall[:, s0:s0 + 1024], xw[:, s0:s0 + 1024])

            def xv(c, col, w):
                # x chunk c (c in 0..1), columns [col, col+w) piece-major
                gp = col // G
                assert col % G + w <= G
                base = W_P0 if gp == 0 else W_CON + (gp - 1) * 1024
                return xall[:, base + c * G + col % G:
                            base + c * G + col % G + w]

            # --- activation buffers ---
            f_t = []
            for gi in range(NIG):
                ft = cpool.tile([33, G], fp8, name=f"f{gi}")
                f_t.append(ft)
                nc.vector.memset(ft[32:33, :], 1.0)
            g_aug = cpool.tile([33, N], fp8)      # rows: g(32), gbv(1)
            hpt = cpool.tile([128, NJT * 33], bf16)
            hpt_v = hpt[:].rearrange("p (t w) -> p t w", w=33)
            # num^T/den accumulators for both ig parities. NOTE: nothing
            # else may live in this bank: any other matmul's start=True
            # would mark the whole 2KB zero-region pending and corrupt
            # the accumulation.
            po_all = psP.tile([128, 2 * 4 * 34], f32, name="po")

            # --- g conv: [33, w] per group (col0/w used to split grp 0
            # during startup so mm1(0) can begin after the first half) ---
            def emit_g_conv(grp, col0=0, w=G):
                cps = psS.tile([33, w], f32, name="s", tag="s") if os.environ.get("KN_GS","0")=="1" else psC.tile([33, w], f32, name="cv", tag="cv")
                for c in range(2):
                    nc.tensor.matmul(
                        cps[:, :],
                        wg_t[:, c * 33:(c + 1) * 33],
                        xv(c, grp * G + col0, w),
                        start=(c == 0), stop=(c == 1))
                nc.vector.tensor_copy(
                    g_aug[:, grp * G + col0:grp * G + col0 + w], cps[:, :])

            # --- h^T conv: 4 j-tiles per group, out [j, 32] directly ---
            def emit_h_conv(grp):
                hps = psS.tile([128, 4 * 32], f32, name="s", tag="s")
                for k in range(4):
                    jt = 4 * grp + k
                    for c in range(2):
                        nc.tensor.matmul(
                            hps[:, k * 32:(k + 1) * 32],
                            xv(c, jt * 128, 128),
                            wqt_t[:, c * 32:(c + 1) * 32],
                            start=(c == 0), stop=(c == 1))
                nc.vector.tensor_copy(
                    hpt_v[:, 4 * grp:4 * grp + 4, 0:32],
                    hps[:, :].rearrange("p (t w) -> p t w", w=32))
                nc.vector.memset(hpt_v[:, 4 * grp:4 * grp + 4, 32:33], 1.0)

            # --- f conv (own query half): f = Wv @ xq, fp8 out ---
            def emit_f_conv(fg):
                cps = psS.tile([32, G], f32, name="s", tag="s")
                for c in range(2):
                    nc.tensor.matmul(
                        cps[:, :],
                        wft_t[:, c * 32:(c + 1) * 32],
                        xv(c, fg * G, G),
                        start=(c == 0), stop=(c == 1))
                nc.vector.tensor_copy(f_t[fg][0:32, :], cps[:, :])

            # --- main attention loop ---
            stages = [(g, si) for g in range(NIG) for si in range(NST)]
            NS = len(stages)

            po_t = {}
            att_t = {}
            sps_t = {}
            eb_t = {}
            rd_t = {}
            op_t = {}
            rt_t = {}

            def emit_mm1(idx):
                g, si = stages[idx]
                nt = SUPERS[si]
                sps = psS.tile([128, SUP * G], f32, name="s")
                sps_t[idx] = sps
                fr = f_t[g][:, :].unsqueeze(1).broadcast_to([33, 2, G])
                for t in range(nt):
                    jt = JT0[si] + t
                    gl = (g_aug[:, jt * 128:(jt + 1) * 128]
                          .unsqueeze(1).broadcast_to([33, 2, 128]))
                    nc.tensor.matmul(
                        sps[:, t * G:(t + 1) * G], gl, fr,
                        start=True, stop=True, perf_mode=DR)

            def emit_exp(idx):
                eng = sched[idx]
                g, si = stages[idx]
                w = SUPERS[si] * G
                eb = epool.tile([128, SUP * G], bf16, name="eb")
                eb_t[idx] = eb
                sps = sps_t.pop(idx)
                if eng == "A":
                    nc.scalar.activation(eb[:, 0:w], sps[:, 0:w], Exp,
                                         scale=0.5)
                else:
                    e = nc.vector if eng == "D" else nc.gpsimd
                    e.tensor_scalar(eb[:, 0:w].bitcast(i16), sps[:, 0:w],
                                    A_SCH, B_SCH, mult, add)
                if DBG and idx == 0:
                    t = rpool.tile([128, SUP * G], f32, name="dbe")
                    nc.vector.tensor_copy(t[:, :], eb[:, :])
                    nc.sync.dma_start(dbg_eb, t[:, :])

            def emit_mm2(idx):
                g, si = stages[idx]
                eb = eb_t.pop(idx)
                if si == 0:
                    po_t[g] = po_all[:, (g % 2) * 136:(g % 2) * 136 + 136]
                for t in range(SUPERS[si]):
                    jt = JT0[si] + t
                    for c in range(4):
                        nc.tensor.matmul(
                            po_t[g][:, c * 34:c * 34 + 33],
                            eb[:, (t * 4 + c) * 128:(t * 4 + c + 1) * 128],
                            hpt_v[:, jt],
                            start=(jt == 0), stop=(jt == NJT - 1))

            # --- tail: scale -> transpose -> out conv (+x via PE) ---
            def emit_tail_scale(g):
                # rden for all 4 chunks in one strided reciprocal, then
                # att^T[i, 0:33] bf16 = po * rden (row 32 -> ~1.0, which
                # doubles as the out-conv bias-ones row)
                rd = spool.tile([128, 4], f32, name="rd")
                pv = po_t[g][:, :].rearrange("p (c w) -> p c w", w=34)
                with nc.allow_low_precision(reason="softmax denom"):
                    nc.vector.reciprocal(rd[:, :], pv[:, :, 32])
                asc = spool.tile([128, 4 * 34], bf16, name="asc")
                nc.vector.tensor_tensor(
                    asc[:, :].rearrange("p (c w) -> p c w", w=34),
                    pv[:, :, :],
                    rd[:, :].unsqueeze(2).broadcast_to([128, 4, 34]),
                    mult)
                rd_t[g] = (rd, asc)
                if DBG and g == 0:
                    t = rpool.tile([128, 136], f32, name="dbp")
                    nc.vector.tensor_copy(t[:, :], po_t[g][:, :])
                    nc.sync.dma_start(dbg_po, t[:, :])
                    t2 = rpool.tile([128, 136], f32, name="dba")
                    nc.vector.tensor_copy(t2[:, :], asc[:, :])
                    nc.sync.dma_start(dbg_asc, t2[:, :])

            def emit_tail_tps(g):
                _, asc = rd_t[g]
                atp = psC.tile([33, 512], bf16, name="cv", tag="cv")
                for c in range(4):
                    nc.tensor.transpose(
                        atp[:, c * 128:(c + 1) * 128],
                        asc[:, c * 34:c * 34 + 33], idm_t)
                att = spool.tile([33, 512], bf16, name="att")
                if os.environ.get("KN_ATTE", "D") == "A":
                    nc.scalar.copy(att[:, :], atp[:, :])
                else:
                    nc.vector.tensor_copy(att[:, :], atp[:, :])
                att_t[g] = att
                if DBG and g == 0:
                    t = rpool.tile([33, 512], f32, name="dbt")
                    nc.vector.tensor_copy(t[:, :], att[:, :])
                    nc.sync.dma_start(dbg_att, t[:, :])

            KN_RESID = os.environ.get("KN_RESID", "pe")

            def emit_tail_conv(g, cc):
                att = att_t[g]
                if cc == 0:
                    op_t[g] = {}
                # per-cc tile in the conv bank (free at tail time) so the
                # psS rotation is never blocked by the tail
                op = psC.tile([128, G], f32, name="cv", tag="cv")
                op_t[g][cc] = op
                nc.tensor.matmul(
                    op[:, :], idr_t, xv(cc, g * G, G),
                    start=True, stop=False, skip_group_check=True)
                nc.tensor.matmul(
                    op[:, :],
                    wot_t[:, cc * 128:(cc + 1) * 128],
                    att[:, :], start=False, stop=True,
                    skip_group_check=True)

            def emit_tail_out(g, cc):
                op = op_t[g][cc]
                if cc == 0:
                    rt_t[g] = rpool.tile([128, 1024], f32, name="rt")
                rt = rt_t[g]
                out_v = res.rearrange("(c p) (gg n) -> p gg c n",
                                      c=2, n=G)[:, g]
                if DBG and g == 0 and cc == 0:
                    t = rpool.tile([128, 1024], f32, name="dbo")
                    nc.vector.tensor_copy(t[:, 0:G], op[:, :])
                    nc.vector.tensor_copy(t[:, G:], op_t[g][0][:, :])
                    nc.sync.dma_start(dbg_op, t[:, :])
                    t2 = rpool.tile([33, 256], f32, name="dbw")
                    nc.vector.tensor_copy(t2[:, :], wot_t[:, :])
                    nc.sync.dma_start(dbg_wot, t2[:, :])
                if g == NIG - 1 and cc == 1 and \
                        os.environ.get("KN_LCA", "0") == "1":
                    nc.scalar.copy(rt[:, cc * G:(cc + 1) * G], op[:, :])
                else:
                    nc.vector.tensor_copy(rt[:, cc * G:(cc + 1) * G],
                                          op[:, :])
                nc.sync.dma_start(out_v[:, cc], rt[:, cc * G:(cc + 1) * G])
                if cc == 1:
                    po_t.pop(g)
                    rd_t.pop(g)
                    att_t.pop(g)
                    op_t.pop(g)
                    rt_t.pop(g)

            # --- pipeline ---
            KN_CAH = int(os.environ.get("KN_CAH", "6"))
            KN_FSI = int(os.environ.get("KN_FSI", str((NST * 11) // 16)))
            KN_TDL = int(os.environ.get("KN_TDL", "1"))
            convs_left = list(range(1, 8))
            f_left = list(range(1, NIG))
            pending = []   # (g, tail_step, stage_done)

            TAIL = [lambda g: emit_tail_scale(g),
                    lambda g: emit_tail_tps(g),
                    lambda g: emit_tail_conv(g, 0),
                    lambda g: emit_tail_conv(g, 1),
                    lambda g: emit_tail_out(g, 0),
                    lambda g: emit_tail_out(g, 1)]
            NTS = len(TAIL)

            # mm2 runs one stage behind (KN_MM2D=1): by the time mm2(k-1)
            # reaches the PE queue head its exp has long finished, so a
            # slow exp never stalls the next stage's mm1 behind it.
            KN_MM2D = int(os.environ.get("KN_MM2D", "1"))
            emit_g_conv(0)
            emit_f_conv(0)
            emit_mm1(0)
            emit_h_conv(0)
            for k in range(NS):
                g, si = stages[k]
                emit_exp(k)
                if k + 1 < NS:
                    emit_mm1(k + 1)
                if k >= KN_MM2D and os.environ.get("KN_M2P", "late") == "early":
                    emit_mm2(k - KN_MM2D)
                if g == 0:
                    need = min((JT0[si] + SUPERS[si] + KN_CAH) // 4, 7)
                    while convs_left and convs_left[0] <= need:
                        gp = convs_left.pop(0)
                        emit_g_conv(gp)
                        emit_h_conv(gp)
                if f_left and si >= KN_FSI and f_left[0] <= g + 1:
                    emit_f_conv(f_left.pop(0))
                if pending and k >= pending[0][2] + KN_TDL:
                    gg, step, _ = pending[0]
                    TAIL[step](gg)
                    if step == NTS - 1:
                        pending.pop(0)
                    else:
                        pending[0] = (gg, step + 1, pending[0][2])
                if k >= KN_MM2D and os.environ.get("KN_M2P", "late") != "early":
                    emit_mm2(k - KN_MM2D)
                if si == NST - 1:
                    pending.append((g, 0, k + KN_MM2D))
            for k in range(max(0, NS - KN_MM2D), NS):
                emit_mm2(k)
            while convs_left:
                gp = convs_left.pop(0)
                emit_g_conv(gp)
                emit_h_conv(gp)
            while f_left:
                emit_f_conv(f_left.pop(0))
            def emit_tail_drain(g):
                # last tail: per-chunk pipeline across DVE/PE so the
                # serial drain chain is as short as possible
                rd = spool.tile([128, 4], f32, name="rd")
                pv = po_t[g][:, :].rearrange("p (c w) -> p c w", w=34)
                asc = spool.tile([128, 4 * 34], bf16, name="asc")
                ascv = asc[:, :].rearrange("p (c w) -> p c w", w=34)
                atp = psC.tile([33, 512], bf16, name="cv", tag="cv")
                att = spool.tile([33, 512], bf16, name="att")
                op = psS.tile([128, 1024], f32, name="s", tag="s")
                rt = rpool.tile([128, 1024], f32, name="rt")
                with nc.allow_low_precision(reason="softmax denom"):
                    nc.vector.reciprocal(rd[:, :], pv[:, :, 32])
                for cc in range(2):
                    nc.tensor.matmul(
                        op[:, cc * G:(cc + 1) * G], idr_t, xv(cc, g * G, G),
                        start=True, stop=False, skip_group_check=True)
                for c in range(4):
                    nc.vector.tensor_tensor(
                        ascv[:, c], pv[:, c],
                        rd[:, c:c + 1].broadcast_to([128, 34]), mult)
                    nc.tensor.transpose(
                        atp[:, c * 128:(c + 1) * 128],
                        asc[:, c * 34:c * 34 + 33], idm_t)
                    nc.vector.tensor_copy(att[:, c * 128:(c + 1) * 128],
                                          atp[:, c * 128:(c + 1) * 128])
                    for cc in range(2):
                        nc.tensor.matmul(
                            op[:, cc * G + c * 128:cc * G + (c + 1) * 128],
                            wot_t[:, cc * 128:(cc + 1) * 128],
                            att[:, c * 128:(c + 1) * 128],
                            start=False, stop=(c == 3),
                            skip_group_check=True)
                out_v = res.rearrange("(c p) (gg n) -> p gg c n",
                                      c=2, n=G)[:, g]
                nc.vector.tensor_copy(rt[:, 0:G], op[:, 0:G])
                nc.sync.dma_start(out_v[:, 0], rt[:, 0:G])
                nc.scalar.copy(rt[:, G:2 * G], op[:, G:2 * G])
                nc.sync.dma_start(out_v[:, 1], rt[:, G:2 * G])
                po_t.pop(g)

            while pending:
                gg, step, _ = pending.pop(0)
                if step == 0 and os.environ.get("KN_DRAIN", "steps") == "pipe":
                    emit_tail_drain(gg)
                else:
                    for st in range(step, NTS):
                        TAIL[st](gg)
            if DBG:
                t = rpool.tile([33, N], f32, name="dbg")
                nc.vector.tensor_copy(t[:, :], g_aug[:, :])
                nc.sync.dma_start(dbg_g, t[:, :])
                t = rpool.tile([33, G], f32, name="dbf")
                nc.vector.tensor_copy(t[:, :], f_t[0][:, :])
                nc.sync.dma_start(dbg_f, t[:, :])
                t = rpool.tile([128, NJT * 33], f32, name="dbh")
                nc.vector.tensor_copy(t[:, :], hpt[:, :])
                nc.sync.dma_start(dbg_h, t[:, :])

    nc.compile()
    return nc


def _host_prep(Wv, bv, Wk, bk, Wq, bq, Wo, bo, gamma):
    import ml_dtypes
    bfd = ml_dtypes.bfloat16
    gam = float(np.asarray(gamma).reshape(-1)[0])

    # g conv lhsT: rows [Wk(32); bv@Wk(1)]
    w_g = np.zeros((33, 256), np.float32)
    w_g[0:32] = Wk
    w_g[32] = bv @ Wk
    wg = np.zeros((128, 66), np.float32)
    for c in range(2):
        wg[:, c * 33:(c + 1) * 33] = w_g.T[c * 128:(c + 1) * 128, :]

    wft = np.zeros((128, 64), np.float32)
    for c in range(2):
        wft[:, c * 32:(c + 1) * 32] = Wv.T[c * 128:(c + 1) * 128, :]

    # out conv lhsT rows k: k<32 -> gamma*Wo^T, k==32 -> bias row (bf16)
    bof = gam * (Wo @ bq + bo)                                  # [256]
    wot = np.zeros((33, 256), np.float32)
    for c in range(2):
        wot[0:32, c * 128:(c + 1) * 128] = gam * Wo[c * 128:(c + 1) * 128, :].T
        wot[32, c * 128:(c + 1) * 128] = bof[c * 128:(c + 1) * 128]

    wqt = np.zeros((128, 64), np.float32)   # bf16 [128, 2x32]
    for c in range(2):
        wqt[:, c * 32:(c + 1) * 32] = Wq.T[c * 128:(c + 1) * 128, :]

    def pack_bf16(a):
        u = a.astype(bfd).view(np.uint16).astype(np.uint32)
        return (u[:, 0::2] | (u[:, 1::2] << 16)).view(np.float32)

    wpk = np.zeros((128, W_CON), np.float32)
    wpk[:, W_WG:W_WG + 66] = wg
    wpk[:, W_WF:W_WF + 64] = wft
    wpk[:, W_WQ:W_WQ + 64] = wqt
    wpk[:, W_IDR:W_IDR + 128] = np.eye(128, dtype=np.float32)
    cpk = np.zeros((128, C_TOT), np.float32)
    cpk[0:33, C_WO:C_WO + 128] = pack_bf16(wot)
    cpk[:, C_ID:C_ID + 64] = pack_bf16(np.eye(128, dtype=np.float32))
    return wpk, cpk


def kernel(**inputs):
    from concourse.bass_utils import run_bass_kernel_spmd

    x = np.asarray(inputs["x"], np.float32)
    consts, cpk = _host_prep(
        np.asarray(inputs["Wv"], np.float32),
        np.asarray(inputs["bv"], np.float32),
        np.asarray(inputs["Wk"], np.float32),
        np.asarray(inputs["bk"], np.float32),
        np.asarray(inputs["Wq"], np.float32),
        np.asarray(inputs["bq"], np.float32),
        np.asarray(inputs["Wo"], np.float32),
        np.asarray(inputs["bo"], np.float32),
        np.asarray(inputs["gamma"], np.float32),
    )

    if "nc" not in _CACHE:
        _CACHE["nc"] = build_program()
    nc = _CACHE["nc"]

    in_maps = []
    for core in range(NCORES):
        b, ih = core // 2, core % 2
        xrot = np.roll(x[b], -ih * NH, axis=1)
        xp = (xrot.reshape(2, 128, 8, 512)
              .transpose(1, 2, 0, 3).reshape(128, 8, 1024))
        xwb = np.empty((128, WTOT), np.float32)
        xwb[:, 0:W_CON] = consts
        xwb[:, W_P0:W_P0 + 1024] = xp[:, 0]
        xwb[:, W_CON:] = xp[:, 1:].reshape(128, 7 * 1024)
        in_maps.append({"xw": xwb, "xc": cpk})

    r = run_bass_kernel_spmd(nc, in_maps, core_ids=list(range(NCORES)),
                             trace=False)
    out = np.empty((B, C, N), np.float32)
    for core in range(NCORES):
        b, ih = core // 2, core % 2
        out[b][:, ih * NH:(ih + 1) * NH] = r.results[core]["res"]
    return out


if __name__ == "__main__":
    nc = build_program()
    print("program built ok")
